# revision 20
# baseline (speedup 1.0000x reference)
"""ConvGCN Trainium2 kernel: 2-layer GCN over 50000 nodes / 800000 edges on 8 NeuronCores.

Strategy (node / graph parallel):
  - Shard nodes contiguously across 8 cores (6250 per core). Weights replicated.
  - Dense phase per core in transposed layout (features on partitions) so no
    f32 DMA-transpose is needed; TensorE is used to transpose back to row
    layout for the gather tables.
  - AllGather of the per-shard transformed node table -> full [50000, *] table
    in every core's DRAM.
  - Edge aggregation: edges bucketed by dst shard (owner core), grouped by
    128-node dst block. Per 128-edge chunk: dma_gather of source rows (512B
    for conv1, 256B for conv2), one-hot(dst_local) built on DVE (4 chunks per
    instruction), TensorE matmul accumulates the segment-sum into PSUM per
    dst block. dma_gather indices are int16, so edges are split into stream A
    (table row < 32768) and stream B (>= 32768, gathered from an offset
    view). Chunk counts per (block, stream) are maxed over cores so all 8
    cores run one identical instruction stream; shorter cores pad with edges
    whose one-hot row is all-zero (dst_local sentinel 300).
  - h = relu(dinv*(agg + xs) + b1); second round (width 64) + AllGather 2;
    final out = dinv*(agg2 + xs2) + b2.
"""

import math

import numpy as np

import concourse.bacc as bacc
import concourse.bass as bass
import concourse.mybir as mybir
import concourse.tile as tile
from concourse.bass_utils import run_bass_kernel_spmd

F32 = mybir.dt.float32
BF16 = mybir.dt.bfloat16
I16 = mybir.dt.int16
U8 = mybir.dt.uint8

FULL_CFG = dict(
    n_nodes=50000,
    n_cores=8,
    f_in=512, f_lat=64,
    i_in=256, i_lat=32,
    g_lat=96, out=64,
)

GATHER_BATCH = 8           # chunks per dma_gather call (>=2048 idx crashes the Q7 ucode)
OH_G = 4                   # chunks per one-hot DVE instruction
SKIP = set()               # timing-ablation: names of instruction classes to omit
SPLIT = 32768              # int16 index limit: stream A rows [0, SPLIT), B [SPLIT, N)


# ----------------------------------------------------------------- host prep

def _prep_edges(src, dst, cfg):
    """Bucket edges by dst shard, sort by dst block, split into int16-safe
    streams, pad each (core, block, stream) to a core-uniform chunk count."""
    n, c = cfg["n_nodes"], cfg["n_cores"]
    shard = n // c
    nb = math.ceil(shard / 128)

    counts = np.zeros((c, nb, 2), dtype=np.int64)
    buckets = [[[None, None] for _ in range(nb)] for _ in range(c)]

    core_of = dst // shard
    np.clip(core_of, 0, c - 1, out=core_of)
    for ci in range(c):
        m = core_of == ci
        s_c, d_c = src[m], dst[m]
        dloc_all = d_c - ci * shard
        blk = dloc_all >> 7
        stream = (s_c >= SPLIT).astype(np.int64)
        order = np.lexsort((s_c, stream, blk))
        s_c, dloc_all, blk, stream = s_c[order], dloc_all[order], blk[order], stream[order]
        for b in range(nb):
            for st in (0, 1):
                m2 = (blk == b) & (stream == st)
                rows = s_c[m2] - (SPLIT if st else 0)
                dl = (dloc_all[m2] - b * 128).astype(np.int16)
                buckets[ci][b][st] = (rows.astype(np.int64), dl)
                counts[ci, b, st] = rows.size

    nchunks = np.maximum(np.ceil(counts / 128.0).astype(np.int64).max(axis=0), 0)
    schedule = []
    for st in (0, 1):
        for b in range(nb):
            if nchunks[b, st] > 0:
                schedule.append((b, st, int(nchunks[b, st])))
    total_chunks = int(sum(g[2] for g in schedule))

    idx_all = np.zeros((c, total_chunks * 128), dtype=np.int16)
    dstloc_all = np.full((c, total_chunks * 128), 300, dtype=np.int16)
    for ci in range(c):
        pos = 0
        for b, st, nch in schedule:
            rows, dl = buckets[ci][b][st]
            k = rows.size
            idx_all[ci, pos:pos + k] = rows
            dstloc_all[ci, pos:pos + k] = dl
            pos += nch * 128
        assert pos == total_chunks * 128

    # dma_gather index layout: logical index i lives at [i % 16, i // 16],
    # replicated across the 8 Q7 cores on device (shipped as 16 partitions)
    idx_wrapped = [idx_all[ci].reshape(-1, 16).T.copy() for ci in range(c)]
    dstloc_wrapped = [dstloc_all[ci].reshape(-1, 128).T.copy() for ci in range(c)]
    return schedule, total_chunks, idx_wrapped, dstloc_wrapped


def _batches(schedule):
    """Static gather batches: consecutive chunks of one stream, <= GATHER_BATCH."""
    out = []
    pos = 0
    cur_stream, cur_start, cur_n = None, 0, 0
    for b, st, nch in schedule:
        for _ in range(nch):
            if cur_stream != st or cur_n == GATHER_BATCH:
                if cur_n:
                    out.append((cur_stream, cur_start, cur_n))
                cur_stream, cur_start, cur_n = st, pos, 0
            cur_n += 1
            pos += 1
    if cur_n:
        out.append((cur_stream, cur_start, cur_n))
    return out


def _chunks512(width):
    out, c0 = [], 0
    while c0 < width:
        w = min(512, width - c0)
        out.append((c0, w))
        c0 += w
    return out


# ------------------------------------------------------------- kernel build

def _build(cfg, schedule, total_chunks, stop_after=None, timing_stub=False):
    n, c = cfg["n_nodes"], cfg["n_cores"]
    shard = n // c
    nb = math.ceil(shard / 128)
    nbf = shard // 128             # full blocks
    rem = shard - nbf * 128        # rows in the partial last block
    F_IN, F_LAT = cfg["f_in"], cfg["f_lat"]
    I_IN, I_LAT = cfg["i_in"], cfg["i_lat"]
    G, OUT = cfg["g_lat"], cfg["out"]
    W1PAD = 128  # conv1 table row width (96 padded to 128 f32 = 512B)

    nc = bacc.Bacc("TRN2", target_bir_lowering=False, debug=False,
                   num_devices=1 if timing_stub else c,
                   num_swdge_queues=4)

    def inp(name, shape, dtype=F32):
        return nc.dram_tensor(name, shape, dtype, kind="ExternalInput")

    featT = inp("featT", [F_IN, shard], BF16)
    imgT = inp("imgT", [I_IN, shard], BF16)
    Wf = inp("Wf", [F_IN, F_LAT], BF16)
    Wi = inp("Wi", [I_IN, I_LAT], BF16)
    W1 = inp("W1", [G, G])
    W2 = inp("W2", [G, OUT])
    bfc = inp("bfc", [F_LAT, 1])
    bic = inp("bic", [I_LAT, 1])
    b1r = inp("b1r", [128, 1, G])
    b2r = inp("b2r", [128, 1, OUT])
    degt = inp("degt", [128, nb])
    iotab_t = inp("iotab", [128, OH_G, 128], I16)
    idx_t = inp("idx", [16, total_chunks * 8], I16)
    dstloc_t = inp("dstloc", [128, total_chunks], I16)

    out_ext = nc.dram_tensor("out", [shard, OUT], BF16, kind="ExternalOutput")

    ag1_in = nc.dram_tensor("ag1_in", [shard, W1PAD], F32)
    table1 = nc.dram_tensor("table1", [n, W1PAD], F32, addr_space="Shared")
    ag2_in = nc.dram_tensor("ag2_in", [shard, OUT], F32)
    table2 = nc.dram_tensor("table2", [n, OUT], F32, addr_space="Shared")

    # chunk index -> (schedule group, first/last flags)
    chunk_group = []
    for gi, (b, st, nch) in enumerate(schedule):
        for j in range(nch):
            chunk_group.append((gi, j == 0, j == nch - 1))

    class _StopBuild(Exception):
        pass

    import contextlib

    with tile.TileContext(nc) as tc, contextlib.suppress(_StopBuild):
        with (
            tc.tile_pool(name="const", bufs=1) as cpool,
            tc.tile_pool(name="persist", bufs=1) as ppool,
            tc.tile_pool(name="psA", bufs=2, space="PSUM") as psA,
            tc.tile_pool(name="psT", bufs=2, space="PSUM") as psT,
        ):
            iotab = cpool.tile_from(iotab_t[:, :, :])
            from concourse.masks import make_identity
            ident = cpool.tile([128, 128], F32, name="ident")
            make_identity(nc, ident[:, :])
            w1 = cpool.tile_from(W1[:, :])
            w2 = cpool.tile_from(W2[:, :])
            b1 = cpool.tile_from(b1r[:, :, :])
            b2 = cpool.tile_from(b2r[:, :, :])
            bf = cpool.tile_from(bfc[:, :])
            bi = cpool.tile_from(bic[:, :])
            idxs = cpool.tile([128, total_chunks * 8], I16, name="idxs")
            for k in range(8):
                nc.sync.dma_start(out=idxs[16 * k:16 * (k + 1), :], in_=idx_t[:, :])
            dstloc = cpool.tile_from(dstloc_t[:, :])
            deg = cpool.tile_from(degt[:, :])

            dinv = cpool.tile([128, nb], F32)
            nc.scalar.sqrt(dinv[:, :], deg[:, :])
            nc.vector.reciprocal(dinv[:, :], dinv[:, :])

            xT = ppool.tile([G, shard], F32, tag="xT")
            xs_own = ppool.tile([128, nb * 128], F32, tag="xsown")
            acc = ppool.tile([128, nb * G], F32, tag="acc")
            nc.vector.memset(xs_own[:, :], 0.0)

            # ---------------- phase A: xT = relu(W.T @ inT + b) ----------------
            # column groups of up to 4*512, K-tiles resident per group
            GRP = []
            g0 = 0
            while g0 < shard:
                gw = min(2048, shard - g0)
                GRP.append((g0, gw))
                g0 += gw
            with tc.tile_pool(name="phA", bufs=3) as fpool, \
                 tc.tile_pool(name="phAw", bufs=1) as wpool, \
                 tc.tile_pool(name="psPh", bufs=1, space="PSUM") as psPh:
                wf_tiles = [wpool.tile_from(Wf[k * 128:(k + 1) * 128, :], name=f"wf{k}")
                            for k in range(F_IN // 128)]
                wi_tiles = [wpool.tile_from(Wi[k * 128:(k + 1) * 128, :], name=f"wi{k}")
                            for k in range(I_IN // 128)]
                for gidx, (g0, gw) in enumerate(GRP):
                    cks = _chunks512(gw)
                    for srcT, wts, lat, off, bias in (
                        (featT, wf_tiles, F_LAT, 0, bf),
                        (imgT, wts_i := wi_tiles, I_LAT, F_LAT, bi),
                    ):
                        pss = [psPh.tile([F_LAT, 512], F32, tag=f"pp{i}",
                                         name=f"pp_{gidx}_{off}_{i}")
                               for i in range(len(cks))]
                        nk = len(wts)
                        for k in range(nk):
                            ft = fpool.tile([128, 2048], BF16, tag="ft")
                            if "featdma" not in SKIP:
                                nc.sync.dma_start(out=ft[:, :gw],
                                                  in_=srcT[k * 128:(k + 1) * 128, g0:g0 + gw])
                            for i, (c0, w) in enumerate(cks):
                                if "featmm" not in SKIP:
                                    nc.tensor.matmul(pss[i][:lat, :w], lhsT=wts[k][:, :],
                                                     rhs=ft[:, c0:c0 + w],
                                                     start=(k == 0), stop=(k == nk - 1))
                        for i, (c0, w) in enumerate(cks):
                            nc.scalar.activation(xT[off:off + lat, g0 + c0:g0 + c0 + w],
                                                 pss[i][:lat, :w],
                                                 mybir.ActivationFunctionType.Relu,
                                                 bias=bias[:, :])

            # ---------------- xwT = W1.T @ xT ; xs rows = dinv * xw ----------------
            xwT = ppool.tile([G, nb * 128], F32, tag="xwT")
            if rem:
                nc.vector.memset(xwT[:, shard:nb * 128], 0.0)
            for c0, w in _chunks512(shard):
                ps = psA.tile([G, 512], F32, tag="ps")
                nc.tensor.matmul(ps[:G, :w], lhsT=w1[:, :], rhs=xT[:, c0:c0 + w],
                                 start=True, stop=True)
                nc.any.tensor_copy(xwT[:, c0:c0 + w], ps[:G, :w])

            for b in range(nb):
                bh = min(128, shard - b * 128)
                pst = psT.tile([128, 128], F32, tag="pst")
                nc.tensor.transpose(pst[:, 0:G], xwT[:, b * 128:b * 128 + 128],
                                    ident[0:G, 0:G])
                nc.vector.tensor_mul(xs_own[0:bh, b * 128:b * 128 + G], pst[0:bh, 0:G],
                                     dinv[0:bh, b:b + 1].to_broadcast([bh, G]))

            xs3 = xs_own[:, :].rearrange("p (b e) -> p b e", e=128)
            nc.sync.dma_start(
                out=ag1_in[0:nbf * 128, :].rearrange("(b p) e -> p b e", p=128),
                in_=xs3[:, 0:nbf, :])
            if rem:
                nc.sync.dma_start(out=ag1_in[nbf * 128:shard, :],
                                  in_=xs_own[0:rem, nbf * 128:(nbf + 1) * 128])

            if stop_after == "phaseA":
                dbg = ppool.tile([128, OUT], BF16, tag="dbg")
                nc.vector.memset(dbg[:, :], 0.0)
                for b in range(nb):
                    bh = min(128, shard - b * 128)
                    nc.sync.dma_start(out=out_ext[b * 128:b * 128 + bh, :],
                                      in_=dbg[0:bh, :])
                raise _StopBuild

            # ---------------- AllGather 1 ----------------
            if timing_stub or "coll" in SKIP:
                nc.sync.dma_start(out=table1[0:shard, :], in_=ag1_in[:, :])
            else:
                nc.gpsimd.collective_compute(
                    "AllGather", mybir.AluOpType.bypass,
                    replica_groups=[list(range(c))],
                    ins=[ag1_in.ap().opt()], outs=[table1.ap().opt()],
                )

            # ---------------- edge aggregation ----------------
            def aggregate(table_ap, width, elem, acc_tile, gtag):
                with tc.tile_pool(name=f"g{gtag}", bufs=4) as gpool, \
                     tc.tile_pool(name=f"oh{gtag}", bufs=4) as opool, \
                     tc.tile_pool(name=f"agg{gtag}", bufs=4, space="PSUM") as pagg:
                    winA = table_ap[0:SPLIT, :]
                    nrows = table_ap.shape[0]
                    winB = table_ap[SPLIT:nrows, :]
                    psum_of_group = {}
                    block_seen = set()
                    for bi, (st, c0, nchb) in enumerate(_batches(schedule)):
                        gt = gpool.tile([128, GATHER_BATCH, elem], F32, tag="gt")
                        win = winB if st else winA
                        if "gather" in SKIP:
                            nc.vector.memset(gt[:, 0:nchb, :], 0.0)
                        elif True:
                            nc.gpsimd.dma_gather(
                                gt[:, 0:nchb, :], win, idxs[:, c0 * 8:(c0 + nchb) * 8],
                                num_idxs=nchb * 128, num_idxs_reg=nchb * 128,
                                elem_size=elem, elem_step=elem, queue_num=bi % 4,
                            )
                        for j0 in range(0, nchb, OH_G):
                            gwc = min(OH_G, nchb - j0)
                            t0 = c0 + j0
                            oh = opool.tile([128, OH_G, 128], F32, tag="oh")
                            if "onehot" not in SKIP:
                                nc.vector.tensor_tensor(
                                    oh[:, 0:gwc, :],
                                    dstloc[:, t0:t0 + gwc].to_broadcast([128, gwc, 128]),
                                    iotab[:, 0:gwc, :], op=mybir.AluOpType.is_equal)
                            for j in range(j0, j0 + gwc):
                                t = c0 + j
                                gi, is_first, is_last = chunk_group[t]
                                b, _st, _n = schedule[gi]
                                if is_first and "aggmm" not in SKIP:
                                    psum_of_group[gi] = pagg.tile(
                                        [128, width], F32, tag="ps",
                                        name=f"aggps{gtag}_{gi}")
                                if "aggmm" not in SKIP:
                                    nc.tensor.matmul(psum_of_group[gi][:, :],
                                                     lhsT=oh[:, j - j0, :],
                                                     rhs=gt[:, j, 0:width],
                                                     start=is_first, stop=is_last)
                                if is_last and "aggmm" not in SKIP:
                                    ps = psum_of_group.pop(gi)
                                    if True:
                                        dstp = acc_tile[:, b * width:(b + 1) * width]
                                        if b not in block_seen:
                                            block_seen.add(b)
                                            nc.any.tensor_copy(dstp, ps[:, :])
                                        else:
                                            nc.any.tensor_add(dstp, dstp, ps[:, :])
                    for b in range(nb):
                        if b not in block_seen:
                            nc.vector.memset(acc_tile[:, b * width:(b + 1) * width], 0.0)

            aggregate(table1.ap(), G, W1PAD, acc, "1")

            # ---------------- h = relu(dinv*(agg+xs) + b1) ----------------
            acc3 = acc[:, :].rearrange("p (b e) -> p b e", e=G)
            nc.vector.tensor_add(acc3, acc3, xs3[:, :, 0:G])
            nc.vector.tensor_mul(acc3, acc3, dinv[:, :].to_broadcast([128, nb, G]))
            nc.vector.tensor_add(acc3, acc3, b1[:, :, :].to_broadcast([128, nb, G]))
            nc.scalar.activation(acc3, acc3, mybir.ActivationFunctionType.Relu)

            if stop_after == "conv1":
                dbg = ppool.tile([128, OUT], BF16, tag="dbg")
                nc.vector.memset(dbg[:, :], 0.0)
                for b in range(nb):
                    bh = min(128, shard - b * 128)
                    nc.sync.dma_start(out=out_ext[b * 128:b * 128 + bh, :],
                                      in_=dbg[0:bh, :])
                raise _StopBuild

            # ---------------- conv2 dense: xs2 = dinv * (h @ W2) ----------------
            hT = ppool.tile([G, nb * 128], F32, tag="hT")
            if rem:
                nc.vector.memset(hT[:, shard:nb * 128], 0.0)
            for b in range(nb):
                pst = psT.tile([G, 128], F32, tag="pst")
                nc.tensor.transpose(pst[:G, :], acc[:, b * G:(b + 1) * G],
                                    ident[:, :])
                nc.any.tensor_copy(hT[:, b * 128:b * 128 + 128], pst[:G, :])

            xs2_own = ppool.tile([128, nb * OUT], F32, tag="xsown2")
            nc.vector.memset(xs2_own[:, :], 0.0)
            hw2T = ppool.tile([OUT, nb * 128], F32, tag="xwT")  # reuse xwT slot
            for c0, w in _chunks512(shard):
                ps = psA.tile([OUT, 512], F32, tag="ps")
                nc.tensor.matmul(ps[:OUT, :w], lhsT=w2[:, :], rhs=hT[:, c0:c0 + w],
                                 start=True, stop=True)
                nc.any.tensor_copy(hw2T[:, c0:c0 + w], ps[:OUT, :w])
            for b in range(nb):
                bh = min(128, shard - b * 128)
                pst = psT.tile([128, OUT], F32, tag="pst")
                nc.tensor.transpose(pst[:, :OUT], hw2T[:, b * 128:b * 128 + 128],
                                    ident[0:OUT, 0:OUT])
                nc.vector.tensor_mul(xs2_own[0:bh, b * OUT:(b + 1) * OUT], pst[0:bh, :OUT],
                                     dinv[0:bh, b:b + 1].to_broadcast([bh, OUT]))

            xs23 = xs2_own[:, :].rearrange("p (b e) -> p b e", e=OUT)
            nc.sync.dma_start(
                out=ag2_in[0:nbf * 128, :].rearrange("(b p) e -> p b e", p=128),
                in_=xs23[:, 0:nbf, :])
            if rem:
                nc.sync.dma_start(out=ag2_in[nbf * 128:shard, :],
                                  in_=xs2_own[0:rem, nbf * OUT:(nbf + 1) * OUT])

            # ---------------- AllGather 2 + conv2 aggregation ----------------
            if timing_stub or "coll" in SKIP:
                nc.sync.dma_start(out=table2[0:shard, :], in_=ag2_in[:, :])
            else:
                nc.gpsimd.collective_compute(
                    "AllGather", mybir.AluOpType.bypass,
                    replica_groups=[list(range(c))],
                    ins=[ag2_in.ap().opt()], outs=[table2.ap().opt()],
                )
            aggregate(table2.ap(), OUT, OUT, acc, "2")

            # ---------------- out = dinv*(agg2+xs2) + b2 ----------------
            acc23 = acc[:, 0:nb * OUT].rearrange("p (b e) -> p b e", e=OUT)
            nc.vector.tensor_add(acc23, acc23, xs23)
            nc.vector.tensor_mul(acc23, acc23, dinv[:, :].to_broadcast([128, nb, OUT]))
            nc.vector.tensor_add(acc23, acc23, b2[:, :, :].to_broadcast([128, nb, OUT]))
            obf = ppool.tile([128, nb * OUT], BF16, tag="obf")
            obf3 = obf[:, :].rearrange("p (b e) -> p b e", e=OUT)
            nc.any.tensor_copy(obf3, acc23)
            nc.sync.dma_start(
                out=out_ext[0:nbf * 128, :].rearrange("(b p) e -> p b e", p=128),
                in_=obf3[:, 0:nbf, :])
            if rem:
                nc.sync.dma_start(out=out_ext[nbf * 128:shard, :],
                                  in_=obf[0:rem, nbf * OUT:(nbf + 1) * OUT])

    nc.compile()
    return nc


# ------------------------------------------------------------------ runner

_CACHE = {}


def _run(inputs, cfg, use_sim=False):
    n, c = cfg["n_nodes"], cfg["n_cores"]
    shard = n // c
    nb = math.ceil(shard / 128)

    ei = np.asarray(inputs["edge_index"]).astype(np.int64)
    src, dst = ei[0], ei[1]
    feature = np.asarray(inputs["feature"], dtype=np.float32)
    img = np.asarray(inputs["img"], dtype=np.float32)

    key = "k"
    if key not in _CACHE:
        schedule, total_chunks, idx_w, dl_w = _prep_edges(src, dst, cfg)
        nc = _build(cfg, schedule, total_chunks, stop_after=cfg.get("stop_after"))
        _CACHE[key] = (nc, schedule, total_chunks, idx_w, dl_w)
    nc, schedule, total_chunks, idx_w, dl_w = _CACHE[key]

    deg = (np.bincount(dst, minlength=n) + 1).astype(np.float32)

    import ml_dtypes
    bf16 = ml_dtypes.bfloat16
    iotab = np.tile(np.arange(128, dtype=np.int16)[None, None, :], (128, OH_G, 1))
    in_maps = []
    for ci in range(c):
        sl = slice(ci * shard, (ci + 1) * shard)
        degp = np.ones(nb * 128, dtype=np.float32)
        degp[:shard] = deg[sl]
        in_maps.append({
            "featT": np.ascontiguousarray(feature[sl].T).astype(bf16),
            "imgT": np.ascontiguousarray(img[sl].T).astype(bf16),
            "Wf": np.asarray(inputs["W_feat"], np.float32).astype(bf16),
            "Wi": np.asarray(inputs["W_img"], np.float32).astype(bf16),
            "W1": np.asarray(inputs["W_g1"], np.float32),
            "W2": np.asarray(inputs["W_g2"], np.float32),
            "bfc": np.asarray(inputs["b_feat"], np.float32)[:, None],
            "bic": np.asarray(inputs["b_img"], np.float32)[:, None],
            "b1r": np.tile(np.asarray(inputs["b_g1"], np.float32)[None, None, :], (128, 1, 1)),
            "b2r": np.tile(np.asarray(inputs["b_g2"], np.float32)[None, None, :], (128, 1, 1)),
            "degt": degp.reshape(nb, 128).T.copy(),
            "iotab": iotab,
            "idx": idx_w[ci],
            "dstloc": dl_w[ci],
        })

    _CACHE["last_in_maps"] = in_maps
    if use_sim:
        from concourse.bass_interp import MultiCoreSim
        sim = MultiCoreSim(nc, c)
        for ci in range(c):
            for k, v in in_maps[ci].items():
                sim.cores[ci].tensor(k)[:] = v
        sim.simulate(check_with_hw=False)
        out = np.concatenate([sim.cores[ci].mem_tensor("out") for ci in range(c)], axis=0)
        return np.asarray(out, dtype=np.float32)
    res = run_bass_kernel_spmd(nc, in_maps, list(range(c)))
    out = np.concatenate([res.results[ci]["out"] for ci in range(c)], axis=0)
    return np.asarray(out, dtype=np.float32)


def kernel(**inputs):
    return _run(inputs, FULL_CFG)



# revision 23
# speedup vs baseline: 14.8094x; 14.8094x over previous
"""ConvGCN Trainium2 kernel: 2-layer GCN over 50000 nodes / 800000 edges on 8 NeuronCores.

Strategy (node / graph parallel):
  - Shard nodes contiguously across 8 cores (6250 per core). Weights replicated.
  - Dense phase per core in transposed layout (features on partitions) so no
    f32 DMA-transpose is needed; TensorE is used to transpose back to row
    layout for the gather tables.
  - AllGather of the per-shard transformed node table -> full [50000, *] table
    in every core's DRAM.
  - Edge aggregation: edges bucketed by dst shard (owner core), grouped by
    128-node dst block. Per 128-edge chunk: dma_gather of source rows (512B
    for conv1, 256B for conv2), one-hot(dst_local) built on DVE (4 chunks per
    instruction), TensorE matmul accumulates the segment-sum into PSUM per
    dst block. dma_gather indices are int16, so edges are split into stream A
    (table row < 32768) and stream B (>= 32768, gathered from an offset
    view). Chunk counts per (block, stream) are maxed over cores so all 8
    cores run one identical instruction stream; shorter cores pad with edges
    whose one-hot row is all-zero (dst_local sentinel 300).
  - h = relu(dinv*(agg + xs) + b1); second round (width 64) + AllGather 2;
    final out = dinv*(agg2 + xs2) + b2.
"""

import math

import numpy as np

import concourse.bacc as bacc
import concourse.bass as bass
import concourse.mybir as mybir
import concourse.tile as tile
from concourse.bass_utils import run_bass_kernel_spmd

F32 = mybir.dt.float32
BF16 = mybir.dt.bfloat16
I16 = mybir.dt.int16
U8 = mybir.dt.uint8

FULL_CFG = dict(
    n_nodes=50000,
    n_cores=8,
    f_in=512, f_lat=64,
    i_in=256, i_lat=32,
    g_lat=96, out=64,
)

GATHER_BATCH = 8           # chunks per dma_gather call (>=2048 idx crashes the Q7 ucode)
OH_G = 4                   # chunks per one-hot DVE instruction
SKIP = set()               # timing-ablation: names of instruction classes to omit
SPLIT = 32768              # int16 index limit: stream A rows [0, SPLIT), B [SPLIT, N)


# ----------------------------------------------------------------- host prep

def _prep_edges(src, dst, cfg):
    """Bucket edges by dst shard, sort by dst block, split into int16-safe
    streams, pad each (core, block, stream) to a core-uniform chunk count."""
    n, c = cfg["n_nodes"], cfg["n_cores"]
    shard = n // c
    nb = math.ceil(shard / 128)

    counts = np.zeros((c, nb, 2), dtype=np.int64)
    buckets = [[[None, None] for _ in range(nb)] for _ in range(c)]

    core_of = dst // shard
    np.clip(core_of, 0, c - 1, out=core_of)
    for ci in range(c):
        m = core_of == ci
        s_c, d_c = src[m], dst[m]
        dloc_all = d_c - ci * shard
        blk = dloc_all >> 7
        stream = (s_c >= SPLIT).astype(np.int64)
        order = np.lexsort((s_c, stream, blk))
        s_c, dloc_all, blk, stream = s_c[order], dloc_all[order], blk[order], stream[order]
        for b in range(nb):
            for st in (0, 1):
                m2 = (blk == b) & (stream == st)
                rows = s_c[m2] - (SPLIT if st else 0)
                dl = (dloc_all[m2] - b * 128).astype(np.int16)
                buckets[ci][b][st] = (rows.astype(np.int64), dl)
                counts[ci, b, st] = rows.size

    nchunks = np.maximum(np.ceil(counts / 128.0).astype(np.int64).max(axis=0), 0)
    schedule = []
    for st in (0, 1):
        for b in range(nb):
            if nchunks[b, st] > 0:
                schedule.append((b, st, int(nchunks[b, st])))
    total_chunks = int(sum(g[2] for g in schedule))

    idx_all = np.zeros((c, total_chunks * 128), dtype=np.int16)
    dstloc_all = np.full((c, total_chunks * 128), 300, dtype=np.int16)
    for ci in range(c):
        pos = 0
        for b, st, nch in schedule:
            rows, dl = buckets[ci][b][st]
            k = rows.size
            idx_all[ci, pos:pos + k] = rows
            dstloc_all[ci, pos:pos + k] = dl
            pos += nch * 128
        assert pos == total_chunks * 128

    # dma_gather index layout: logical index i lives at [i % 16, i // 16],
    # replicated across the 8 Q7 cores on device (shipped as 16 partitions)
    idx_wrapped = [idx_all[ci].reshape(-1, 16).T.copy() for ci in range(c)]
    dstloc_wrapped = [dstloc_all[ci].reshape(-1, 128).T.copy() for ci in range(c)]
    return schedule, total_chunks, idx_wrapped, dstloc_wrapped


def _batches(schedule):
    """Static gather batches: consecutive chunks of one stream, <= GATHER_BATCH."""
    out = []
    pos = 0
    cur_stream, cur_start, cur_n = None, 0, 0
    for b, st, nch in schedule:
        for _ in range(nch):
            if cur_stream != st or cur_n == GATHER_BATCH:
                if cur_n:
                    out.append((cur_stream, cur_start, cur_n))
                cur_stream, cur_start, cur_n = st, pos, 0
            cur_n += 1
            pos += 1
    if cur_n:
        out.append((cur_stream, cur_start, cur_n))
    return out


def _chunks512(width):
    out, c0 = [], 0
    while c0 < width:
        w = min(512, width - c0)
        out.append((c0, w))
        c0 += w
    return out


# ------------------------------------------------------------- kernel build

def _build(cfg, schedule, total_chunks, stop_after=None, timing_stub=False,
           repeats=1):
    n, c = cfg["n_nodes"], cfg["n_cores"]
    shard = n // c
    nb = math.ceil(shard / 128)
    nbf = shard // 128             # full blocks
    rem = shard - nbf * 128        # rows in the partial last block
    F_IN, F_LAT = cfg["f_in"], cfg["f_lat"]
    I_IN, I_LAT = cfg["i_in"], cfg["i_lat"]
    G, OUT = cfg["g_lat"], cfg["out"]
    W1PAD = 128  # conv1 table row width (96 padded to 128 f32 = 512B)

    nc = bacc.Bacc("TRN2", target_bir_lowering=False, debug=False,
                   num_devices=1 if timing_stub else c,
                   num_swdge_queues=4)

    def inp(name, shape, dtype=F32):
        return nc.dram_tensor(name, shape, dtype, kind="ExternalInput")

    featT = inp("featT", [F_IN, shard], BF16)
    imgT = inp("imgT", [I_IN, shard], BF16)
    Wf = inp("Wf", [F_IN, F_LAT], BF16)
    Wi = inp("Wi", [I_IN, I_LAT], BF16)
    W1 = inp("W1", [G, G])
    W2 = inp("W2", [G, OUT])
    bfc = inp("bfc", [F_LAT, 1])
    bic = inp("bic", [I_LAT, 1])
    b1r = inp("b1r", [128, 1, G])
    b2r = inp("b2r", [128, 1, OUT])
    degt = inp("degt", [128, nb])
    iotab_t = inp("iotab", [128, OH_G, 128], I16)
    idx_t = inp("idx", [16, total_chunks * 8], I16)
    dstloc_t = inp("dstloc", [128, total_chunks], I16)

    out_ext = nc.dram_tensor("out", [shard, OUT], BF16, kind="ExternalOutput")

    ag1_in = nc.dram_tensor("ag1_in", [shard, W1PAD], F32)
    table1 = nc.dram_tensor("table1", [n, W1PAD], F32, addr_space="Shared")
    ag2_in = nc.dram_tensor("ag2_in", [shard, OUT], F32)
    table2 = nc.dram_tensor("table2", [n, OUT], F32, addr_space="Shared")

    # chunk index -> (schedule group, first/last flags)
    chunk_group = []
    for gi, (b, st, nch) in enumerate(schedule):
        for j in range(nch):
            chunk_group.append((gi, j == 0, j == nch - 1))

    class _StopBuild(Exception):
        pass

    import contextlib

    with tile.TileContext(nc) as tc, contextlib.suppress(_StopBuild):
      for _rep in range(repeats):
        sfx = f"_r{_rep}" if repeats > 1 else ""
        with (
            tc.tile_pool(name="const" + sfx, bufs=1) as cpool,
            tc.tile_pool(name="persist" + sfx, bufs=1) as ppool,
            tc.tile_pool(name="psA" + sfx, bufs=2, space="PSUM") as psA,
            tc.tile_pool(name="psT" + sfx, bufs=2, space="PSUM") as psT,
        ):
            iotab = cpool.tile_from(iotab_t[:, :, :])
            from concourse.masks import make_identity
            ident = cpool.tile([128, 128], F32, name="ident" + sfx)
            make_identity(nc, ident[:, :])
            w1 = cpool.tile_from(W1[:, :])
            w2 = cpool.tile_from(W2[:, :])
            b1 = cpool.tile_from(b1r[:, :, :])
            b2 = cpool.tile_from(b2r[:, :, :])
            bf = cpool.tile_from(bfc[:, :])
            bi = cpool.tile_from(bic[:, :])
            idxs = cpool.tile([128, total_chunks * 8], I16, name="idxs" + sfx)
            for k in range(8):
                nc.sync.dma_start(out=idxs[16 * k:16 * (k + 1), :], in_=idx_t[:, :])
            dstloc = cpool.tile_from(dstloc_t[:, :])
            deg = cpool.tile_from(degt[:, :])

            dinv = cpool.tile([128, nb], F32)
            nc.scalar.sqrt(dinv[:, :], deg[:, :])
            nc.vector.reciprocal(dinv[:, :], dinv[:, :])

            xT = ppool.tile([G, shard], F32, tag="xT")
            xs_own = ppool.tile([128, nb * 128], F32, tag="xsown")
            acc = ppool.tile([128, nb * G], F32, tag="acc")
            nc.vector.memset(xs_own[:, :], 0.0)

            # ---------------- phase A: xT = relu(W.T @ inT + b) ----------------
            # column groups of up to 4*512, K-tiles resident per group
            GRP = []
            g0 = 0
            while g0 < shard:
                gw = min(2048, shard - g0)
                GRP.append((g0, gw))
                g0 += gw
            with tc.tile_pool(name="phA" + sfx, bufs=3) as fpool, \
                 tc.tile_pool(name="phAw" + sfx, bufs=1) as wpool, \
                 tc.tile_pool(name="psPh" + sfx, bufs=1, space="PSUM") as psPh:
                wf_tiles = [wpool.tile_from(Wf[k * 128:(k + 1) * 128, :], name=f"wf{k}{sfx}")
                            for k in range(F_IN // 128)]
                wi_tiles = [wpool.tile_from(Wi[k * 128:(k + 1) * 128, :], name=f"wi{k}{sfx}")
                            for k in range(I_IN // 128)]
                for gidx, (g0, gw) in enumerate(GRP):
                    cks = _chunks512(gw)
                    for srcT, wts, lat, off, bias in (
                        (featT, wf_tiles, F_LAT, 0, bf),
                        (imgT, wts_i := wi_tiles, I_LAT, F_LAT, bi),
                    ):
                        pss = [psPh.tile([F_LAT, 512], F32, tag=f"pp{i}",
                                         name=f"pp{sfx}_{gidx}_{off}_{i}")
                               for i in range(len(cks))]
                        nk = len(wts)
                        for k in range(nk):
                            ft = fpool.tile([128, 2048], BF16, tag="ft")
                            if "featdma" not in SKIP:
                                nc.sync.dma_start(out=ft[:, :gw],
                                                  in_=srcT[k * 128:(k + 1) * 128, g0:g0 + gw])
                            for i, (c0, w) in enumerate(cks):
                                if "featmm" not in SKIP:
                                    nc.tensor.matmul(pss[i][:lat, :w], lhsT=wts[k][:, :],
                                                     rhs=ft[:, c0:c0 + w],
                                                     start=(k == 0), stop=(k == nk - 1))
                        for i, (c0, w) in enumerate(cks):
                            nc.scalar.activation(xT[off:off + lat, g0 + c0:g0 + c0 + w],
                                                 pss[i][:lat, :w],
                                                 mybir.ActivationFunctionType.Relu,
                                                 bias=bias[:, :])

            # ---------------- xwT = W1.T @ xT ; xs rows = dinv * xw ----------------
            xwT = ppool.tile([G, nb * 128], F32, tag="xwT")
            if rem:
                nc.vector.memset(xwT[:, shard:nb * 128], 0.0)
            for c0, w in _chunks512(shard):
                ps = psA.tile([G, 512], F32, tag="ps")
                nc.tensor.matmul(ps[:G, :w], lhsT=w1[:, :], rhs=xT[:, c0:c0 + w],
                                 start=True, stop=True)
                nc.any.tensor_copy(xwT[:, c0:c0 + w], ps[:G, :w])

            for b in range(nb):
                bh = min(128, shard - b * 128)
                pst = psT.tile([128, 128], F32, tag="pst")
                nc.tensor.transpose(pst[:, 0:G], xwT[:, b * 128:b * 128 + 128],
                                    ident[0:G, 0:G])
                nc.vector.tensor_mul(xs_own[0:bh, b * 128:b * 128 + G], pst[0:bh, 0:G],
                                     dinv[0:bh, b:b + 1].to_broadcast([bh, G]))

            xs3 = xs_own[:, :].rearrange("p (b e) -> p b e", e=128)
            nc.sync.dma_start(
                out=ag1_in[0:nbf * 128, :].rearrange("(b p) e -> p b e", p=128),
                in_=xs3[:, 0:nbf, :])
            if rem:
                nc.sync.dma_start(out=ag1_in[nbf * 128:shard, :],
                                  in_=xs_own[0:rem, nbf * 128:(nbf + 1) * 128])

            if stop_after == "phaseA":
                dbg = ppool.tile([128, OUT], BF16, tag="dbg")
                nc.vector.memset(dbg[:, :], 0.0)
                for b in range(nb):
                    bh = min(128, shard - b * 128)
                    nc.sync.dma_start(out=out_ext[b * 128:b * 128 + bh, :],
                                      in_=dbg[0:bh, :])
                raise _StopBuild

            # ---------------- AllGather 1 ----------------
            if timing_stub or "coll" in SKIP:
                nc.sync.dma_start(out=table1[0:shard, :], in_=ag1_in[:, :])
            else:
                nc.gpsimd.collective_compute(
                    "AllGather", mybir.AluOpType.bypass,
                    replica_groups=[list(range(c))],
                    ins=[ag1_in.ap().opt()], outs=[table1.ap().opt()],
                )

            # ---------------- edge aggregation ----------------
            def aggregate(table_ap, width, elem, acc_tile, gtag):
                with tc.tile_pool(name=f"g{gtag}", bufs=4) as gpool, \
                     tc.tile_pool(name=f"oh{gtag}", bufs=4) as opool, \
                     tc.tile_pool(name=f"agg{gtag}", bufs=4, space="PSUM") as pagg:
                    winA = table_ap[0:SPLIT, :]
                    nrows = table_ap.shape[0]
                    winB = table_ap[SPLIT:nrows, :]
                    psum_of_group = {}
                    block_seen = set()
                    for bi, (st, c0, nchb) in enumerate(_batches(schedule)):
                        gt = gpool.tile([128, GATHER_BATCH, elem], F32, tag="gt")
                        win = winB if st else winA
                        if "gather" in SKIP:
                            nc.vector.memset(gt[:, 0:nchb, :], 0.0)
                        elif True:
                            nc.gpsimd.dma_gather(
                                gt[:, 0:nchb, :], win, idxs[:, c0 * 8:(c0 + nchb) * 8],
                                num_idxs=nchb * 128, num_idxs_reg=nchb * 128,
                                elem_size=elem, elem_step=elem, queue_num=bi % 4,
                            )
                        for j0 in range(0, nchb, OH_G):
                            gwc = min(OH_G, nchb - j0)
                            t0 = c0 + j0
                            oh = opool.tile([128, OH_G, 128], F32, tag="oh")
                            if "onehot" not in SKIP:
                                nc.vector.tensor_tensor(
                                    oh[:, 0:gwc, :],
                                    dstloc[:, t0:t0 + gwc].to_broadcast([128, gwc, 128]),
                                    iotab[:, 0:gwc, :], op=mybir.AluOpType.is_equal)
                            for j in range(j0, j0 + gwc):
                                t = c0 + j
                                gi, is_first, is_last = chunk_group[t]
                                b, _st, _n = schedule[gi]
                                if is_first and "aggmm" not in SKIP:
                                    psum_of_group[gi] = pagg.tile(
                                        [128, width], F32, tag="ps",
                                        name=f"aggps{gtag}_{gi}")
                                if "aggmm" not in SKIP:
                                    nc.tensor.matmul(psum_of_group[gi][:, :],
                                                     lhsT=oh[:, j - j0, :],
                                                     rhs=gt[:, j, 0:width],
                                                     start=is_first, stop=is_last)
                                if is_last and "aggmm" not in SKIP:
                                    ps = psum_of_group.pop(gi)
                                    if True:
                                        dstp = acc_tile[:, b * width:(b + 1) * width]
                                        if b not in block_seen:
                                            block_seen.add(b)
                                            nc.any.tensor_copy(dstp, ps[:, :])
                                        else:
                                            nc.any.tensor_add(dstp, dstp, ps[:, :])
                    for b in range(nb):
                        if b not in block_seen:
                            nc.vector.memset(acc_tile[:, b * width:(b + 1) * width], 0.0)

            aggregate(table1.ap(), G, W1PAD, acc, "1" + sfx)

            # ---------------- h = relu(dinv*(agg+xs) + b1) ----------------
            acc3 = acc[:, :].rearrange("p (b e) -> p b e", e=G)
            nc.vector.tensor_add(acc3, acc3, xs3[:, :, 0:G])
            nc.vector.tensor_mul(acc3, acc3, dinv[:, :].to_broadcast([128, nb, G]))
            nc.vector.tensor_add(acc3, acc3, b1[:, :, :].to_broadcast([128, nb, G]))
            nc.scalar.activation(acc3, acc3, mybir.ActivationFunctionType.Relu)

            if stop_after == "conv1":
                dbg = ppool.tile([128, OUT], BF16, tag="dbg")
                nc.vector.memset(dbg[:, :], 0.0)
                for b in range(nb):
                    bh = min(128, shard - b * 128)
                    nc.sync.dma_start(out=out_ext[b * 128:b * 128 + bh, :],
                                      in_=dbg[0:bh, :])
                raise _StopBuild

            # ---------------- conv2 dense: xs2 = dinv * (h @ W2) ----------------
            hT = ppool.tile([G, nb * 128], F32, tag="hT")
            if rem:
                nc.vector.memset(hT[:, shard:nb * 128], 0.0)
            for b in range(nb):
                pst = psT.tile([G, 128], F32, tag="pst")
                nc.tensor.transpose(pst[:G, :], acc[:, b * G:(b + 1) * G],
                                    ident[:, :])
                nc.any.tensor_copy(hT[:, b * 128:b * 128 + 128], pst[:G, :])

            xs2_own = ppool.tile([128, nb * OUT], F32, tag="xsown2")
            nc.vector.memset(xs2_own[:, :], 0.0)
            hw2T = ppool.tile([OUT, nb * 128], F32, tag="xwT")  # reuse xwT slot
            for c0, w in _chunks512(shard):
                ps = psA.tile([OUT, 512], F32, tag="ps")
                nc.tensor.matmul(ps[:OUT, :w], lhsT=w2[:, :], rhs=hT[:, c0:c0 + w],
                                 start=True, stop=True)
                nc.any.tensor_copy(hw2T[:, c0:c0 + w], ps[:OUT, :w])
            for b in range(nb):
                bh = min(128, shard - b * 128)
                pst = psT.tile([128, OUT], F32, tag="pst")
                nc.tensor.transpose(pst[:, :OUT], hw2T[:, b * 128:b * 128 + 128],
                                    ident[0:OUT, 0:OUT])
                nc.vector.tensor_mul(xs2_own[0:bh, b * OUT:(b + 1) * OUT], pst[0:bh, :OUT],
                                     dinv[0:bh, b:b + 1].to_broadcast([bh, OUT]))

            xs23 = xs2_own[:, :].rearrange("p (b e) -> p b e", e=OUT)
            nc.sync.dma_start(
                out=ag2_in[0:nbf * 128, :].rearrange("(b p) e -> p b e", p=128),
                in_=xs23[:, 0:nbf, :])
            if rem:
                nc.sync.dma_start(out=ag2_in[nbf * 128:shard, :],
                                  in_=xs2_own[0:rem, nbf * OUT:(nbf + 1) * OUT])

            # ---------------- AllGather 2 + conv2 aggregation ----------------
            if timing_stub or "coll" in SKIP:
                nc.sync.dma_start(out=table2[0:shard, :], in_=ag2_in[:, :])
            else:
                nc.gpsimd.collective_compute(
                    "AllGather", mybir.AluOpType.bypass,
                    replica_groups=[list(range(c))],
                    ins=[ag2_in.ap().opt()], outs=[table2.ap().opt()],
                )
            aggregate(table2.ap(), OUT, OUT, acc, "2" + sfx)

            # ---------------- out = dinv*(agg2+xs2) + b2 ----------------
            acc23 = acc[:, 0:nb * OUT].rearrange("p (b e) -> p b e", e=OUT)
            nc.vector.tensor_add(acc23, acc23, xs23)
            nc.vector.tensor_mul(acc23, acc23, dinv[:, :].to_broadcast([128, nb, OUT]))
            nc.vector.tensor_add(acc23, acc23, b2[:, :, :].to_broadcast([128, nb, OUT]))
            obf = ppool.tile([128, nb * OUT], BF16, tag="obf")
            obf3 = obf[:, :].rearrange("p (b e) -> p b e", e=OUT)
            nc.any.tensor_copy(obf3, acc23)
            nc.sync.dma_start(
                out=out_ext[0:nbf * 128, :].rearrange("(b p) e -> p b e", p=128),
                in_=obf3[:, 0:nbf, :])
            if rem:
                nc.sync.dma_start(out=out_ext[nbf * 128:shard, :],
                                  in_=obf[0:rem, nbf * OUT:(nbf + 1) * OUT])

    nc.compile()
    return nc


# ------------------------------------------------------------------ runner

_CACHE = {}


def _run(inputs, cfg, use_sim=False):
    n, c = cfg["n_nodes"], cfg["n_cores"]
    shard = n // c
    nb = math.ceil(shard / 128)

    ei = np.asarray(inputs["edge_index"]).astype(np.int64)
    src, dst = ei[0], ei[1]
    feature = np.asarray(inputs["feature"], dtype=np.float32)
    img = np.asarray(inputs["img"], dtype=np.float32)

    key = "k"
    if key not in _CACHE:
        schedule, total_chunks, idx_w, dl_w = _prep_edges(src, dst, cfg)
        nc = _build(cfg, schedule, total_chunks, stop_after=cfg.get("stop_after"))
        _CACHE[key] = (nc, schedule, total_chunks, idx_w, dl_w)
    nc, schedule, total_chunks, idx_w, dl_w = _CACHE[key]

    deg = (np.bincount(dst, minlength=n) + 1).astype(np.float32)

    import ml_dtypes
    bf16 = ml_dtypes.bfloat16
    iotab = np.tile(np.arange(128, dtype=np.int16)[None, None, :], (128, OH_G, 1))
    in_maps = []
    for ci in range(c):
        sl = slice(ci * shard, (ci + 1) * shard)
        degp = np.ones(nb * 128, dtype=np.float32)
        degp[:shard] = deg[sl]
        in_maps.append({
            "featT": np.ascontiguousarray(feature[sl].T).astype(bf16),
            "imgT": np.ascontiguousarray(img[sl].T).astype(bf16),
            "Wf": np.asarray(inputs["W_feat"], np.float32).astype(bf16),
            "Wi": np.asarray(inputs["W_img"], np.float32).astype(bf16),
            "W1": np.asarray(inputs["W_g1"], np.float32),
            "W2": np.asarray(inputs["W_g2"], np.float32),
            "bfc": np.asarray(inputs["b_feat"], np.float32)[:, None],
            "bic": np.asarray(inputs["b_img"], np.float32)[:, None],
            "b1r": np.tile(np.asarray(inputs["b_g1"], np.float32)[None, None, :], (128, 1, 1)),
            "b2r": np.tile(np.asarray(inputs["b_g2"], np.float32)[None, None, :], (128, 1, 1)),
            "degt": degp.reshape(nb, 128).T.copy(),
            "iotab": iotab,
            "idx": idx_w[ci],
            "dstloc": dl_w[ci],
        })

    _CACHE["last_in_maps"] = in_maps
    if use_sim:
        from concourse.bass_interp import MultiCoreSim
        sim = MultiCoreSim(nc, c)
        for ci in range(c):
            for k, v in in_maps[ci].items():
                sim.cores[ci].tensor(k)[:] = v
        sim.simulate(check_with_hw=False)
        out = np.concatenate([sim.cores[ci].mem_tensor("out") for ci in range(c)], axis=0)
        return np.asarray(out, dtype=np.float32)
    res = run_bass_kernel_spmd(nc, in_maps, list(range(c)))
    out = np.concatenate([res.results[ci]["out"] for ci in range(c)], axis=0)
    return np.asarray(out, dtype=np.float32)


def kernel(**inputs):
    return _run(inputs, FULL_CFG)

